# revision 2
# baseline (speedup 1.0000x reference)
"""GCN 2-layer + link decode on 8 TRN2 NeuronCores (full inputs in/out). v2.

Measured-constant-driven redesign of the dest-sharded scatter-free design:
- fp16 tables/stages/sels (256B gather rows; PE 1cyc/row; DVE 2x).
- A^T orientation: psum[feat, slot] += stage_blk(lhsT, stationary, reused)
  @ sel(rhs); windows of 8 chunks accumulate into 2 PSUM banks [128,512];
  retire: AT->sbuf (ACT), hT = W1^T@AT (PE), relu (ACT), per-chunk transpose
  back to row-major h (PE+ACT), DMA fp16.
- h/uv tables split into 4 AllGather pieces == int16 gather ranges, so
  piece r's collective fires as soon as its chunks retire and layer-2 /
  decode gathers on range r depend only on piece r.
- Decode: no routing at all. Pairs sorted by (range(t0), range(t1)); u and v
  gathered in pair order (4 + 16 calls); out = u[:,0:2] + v[:,2:4] via one
  DVE add; host unshuffles.
- Engine split: gpsimd=dma_gather gen only; DVE=sel builds; ACT=copies/relu;
  PE=matmuls/transposes; sync=static DMA.
"""
import numpy as np

P = 128
N = 100_000
NSHARD = 12_500
SLOTS = 12_544
CHUNKS = SLOTS // P          # 98
NCORES = 8
W = 4                        # chunks per window == cells per gather call
NW = (CHUNKS + W - 1) // W   # 13

# x table: raw node rows, 4 ranges of 32768
XTAB = 100_352
XRLO = [0, 32768, 65536, 98304]
XRHI = [32768, 65536, 98304, 100352]

# h/uv tables: pieced (core,slot) mapping; piece p = slots [HS0[p], HS0[p+1])
# table row = PBASE[p] + core*HSLOT[p] + (slot - HS0[p])
HS0 = [0, 3200, 6400, 9600, 12544]
HSLOT = [3200, 3200, 3200, 2944]
PBASE = [0, 25600, 51200, 76800]
HTAB = 100_352
PCHUNK = [0, 25, 50, 75, 98]     # piece p covers chunks [PCHUNK[p], PCHUNK[p+1])


def _node_row(n):
    """h/uv table row for raw node id n."""
    c = n // NSHARD
    s = n - c * NSHARD
    p = np.minimum(s // 3200, 3)
    return (np.array(PBASE)[p] + c * np.array(HSLOT)[p] + (s - np.array(HS0)[p]))


def _wrap_idx(a):
    """[NCORES, T] int16 -> [NCORES, 128, T//16] (16-wrap, 8x replicate)."""
    ncr, t = a.shape
    out = a.reshape(ncr, t // 16, 16).transpose(0, 2, 1)
    return np.ascontiguousarray(np.tile(out, (1, 8, 1)))


def _prep_stream(tab_row, slot, w, rlo, rhi):
    """Edge streams, cell=(range,chunk) padded to cross-core max, 8-chunk
    windows; block-major schedule for A^T-orientation sel matmuls.

    tab_row/slot/w: lists of per-core arrays. Returns layout, sched, idx16,
    rel, wgt.  sched[(r, w)] = list of (blk, k_local, selcol) block-major.
    """
    ncr = len(tab_row)
    nrange = len(rlo)
    rbound = np.asarray(rhi[:-1])

    def range_of(a):
        return np.searchsorted(rbound, a, side="right")

    counts = np.zeros((ncr, nrange, CHUNKS), np.int64)
    for c in range(ncr):
        ch = slot[c] // P
        np.add.at(counts, (c, range_of(tab_row[c]), ch), 1)
    estar = counts.max(axis=0)                      # [nrange, CHUNKS]

    layout = []
    for r in range(nrange):
        calls = []
        base = 0
        for wi in range(NW):
            k0, k1 = wi * W, min(wi * W + W, CHUNKS)
            cells = estar[r, k0:k1]
            offs = np.concatenate([[0], np.cumsum(cells)]).astype(np.int64)
            n = int(offs[-1])
            n_pad = max(P, ((n + P - 1) // P) * P)
            calls.append(dict(k0=k0, k1=k1, offs=offs, n=n, n_pad=n_pad,
                              base=base))
            base += n_pad
        layout.append(dict(calls=calls, T=base))

    sched = {}
    selmap = {}
    n_sel = 0
    for wi in range(NW):
        for r in range(nrange):
            call = layout[r]["calls"][wi]
            offs, k0 = call["offs"], call["k0"]
            lst = []
            for b in range(call["n_pad"] // P):
                e0, e1 = b * P, b * P + P
                ks = [k for k in range(call["k0"], call["k1"])
                      if offs[k - k0] < e1 and offs[k - k0 + 1] > e0]
                if not ks:
                    ks = [call["k0"]]
                for k in ks:
                    lst.append((b, k - call["k0"], n_sel))
                    selmap[(r, wi, b, k)] = n_sel
                    n_sel += 1
            sched[(r, wi)] = lst

    idx16 = [np.zeros((ncr, layout[r]["T"]), np.int16) for r in range(nrange)]
    rel = np.zeros((ncr, P, n_sel), np.float32)
    wgt = np.zeros((ncr, P, n_sel), np.float32)
    for c in range(ncr):
        tr, sl, ww = tab_row[c], slot[c], w[c]
        rr = range_of(tr)
        ch = sl // P
        o = np.lexsort((sl, ch, rr))
        tr, sl, ww, rr, ch = tr[o], sl[o], ww[o], rr[o], ch[o]
        for r in range(nrange):
            m = rr == r
            if not m.any():
                continue
            trm, slm, wwm, chm = tr[m], sl[m], ww[m], ch[m]
            cell_cnt = np.zeros(CHUNKS, np.int64)
            np.add.at(cell_cnt, chm, 1)
            cstart = np.concatenate([[0], np.cumsum(cell_cnt)])
            within = np.arange(len(slm)) - cstart[chm]
            wid = chm // W
            calls = layout[r]["calls"]
            cbase = np.array([cl["base"] for cl in calls], np.int64)
            cell_off = np.zeros(CHUNKS, np.int64)
            for wi, cl in enumerate(calls):
                for k in range(cl["k0"], cl["k1"]):
                    cell_off[k] = cl["offs"][k - cl["k0"]]
            pos = cbase[wid] + cell_off[chm] + within
            idx16[r][c, pos] = (trm - rlo[r]).astype(np.int16)
            relpos = pos - cbase[wid]
            blk = relpos // P
            pp = relpos % P
            cols = np.array([selmap[(r, int(w_), int(b_), int(k_))]
                             for w_, b_, k_ in zip(wid, blk, chm)], np.int64)
            rel[c, pp, cols] = (slm % P).astype(np.float32)
            wgt[c, pp, cols] = wwm
    return dict(layout=layout, sched=sched, n_sel=n_sel, idx16=idx16,
                rel=rel, wgt=wgt)


def _prep_decode(t0_row, t1_row):
    """Canonical per-core pair streams: sort by (r0, r1, orig); 16 segments
    padded to 128-mult and cross-core max. Returns segment table, idx arrays
    (u: per r0; v: per (r0, r1)), and per-core position of each pair."""
    ncr = len(t0_row)
    r0 = [np.minimum(t // 25600, 3) for t in t0_row]
    r1 = [np.minimum(t // 25600, 3) for t in t1_row]
    seglen = np.zeros((16,), np.int64)
    for c in range(ncr):
        cnt = np.bincount((r0[c] * 4 + r1[c]).astype(np.int64), minlength=16)
        seglen = np.maximum(seglen, cnt)
    seglen = ((seglen + P - 1) // P) * P
    segoff = np.concatenate([[0], np.cumsum(seglen)]).astype(np.int64)
    T = int(segoff[-1])

    idx_u = np.zeros((ncr, T), np.int16)
    idx_v = np.zeros((ncr, T), np.int16)
    pos_of = []
    for c in range(ncr):
        key = r0[c] * 4 + r1[c]
        o = np.lexsort((np.arange(len(key)), key))
        pos = np.empty(len(key), np.int64)
        cnt = np.zeros(16, np.int64)
        ks = key[o]
        within = np.zeros(len(key), np.int64)
        for i, k in enumerate(ks):
            within[i] = cnt[k]
            cnt[k] += 1
        pos[o] = segoff[ks] + within
        pos_of.append(pos)
        idx_u[c, pos] = (t0_row[c] - np.array(PBASE)[r0[c]]).astype(np.int16)
        idx_v[c, pos] = (t1_row[c] - np.array(PBASE)[r1[c]]).astype(np.int16)
    return dict(seglen=seglen, segoff=segoff, T=T, idx_u=idx_u, idx_v=idx_v,
                pos_of=pos_of)


def kernel(x, edge_index1, edge_index2, edge_weight1, edge_weight2,
           pos_edge_index, W1, W2, Wlin):
    import concourse.bass as bass
    from concourse import bacc, tile, mybir
    from concourse.bass_utils import run_bass_kernel_spmd
    from concourse.library_config import mlp
    from concourse.masks import make_identity

    f32, f16, i16 = mybir.dt.float32, mybir.dt.float16, mybir.dt.int16
    x = np.asarray(x, np.float32)
    W1 = np.asarray(W1, np.float32)
    W2 = np.asarray(W2, np.float32)
    Wlin = np.asarray(Wlin, np.float32)
    e1 = np.asarray(edge_index1).astype(np.int64)
    e2 = np.asarray(edge_index2).astype(np.int64)
    w1 = np.asarray(edge_weight1, np.float32)
    w2 = np.asarray(edge_weight2, np.float32)
    pe = np.asarray(pos_edge_index).astype(np.int64)

    # ---------- host index preprocessing ----------
    x_tab = np.zeros((XTAB, P), np.float16)
    x_tab[:N] = x.astype(np.float16)

    def shard_by_dest(src_rows, dst, w):
        owner = dst // NSHARD
        ld = dst - owner * NSHARD
        return ([src_rows[owner == c] for c in range(NCORES)],
                [ld[owner == c] for c in range(NCORES)],
                [w[owner == c] for c in range(NCORES)])

    l1 = _prep_stream(*shard_by_dest(e1[0], e1[1], w1), XRLO, XRHI)
    l2 = _prep_stream(*shard_by_dest(_node_row(e2[0]), e2[1], w2),
                      PBASE, PBASE[1:] + [HTAB])

    npairs = pe.shape[1]
    pershard = npairs // NCORES
    t0r = _node_row(pe[0])
    t1r = _node_row(pe[1])
    dec = _prep_decode([t0r[c * pershard:(c + 1) * pershard] for c in range(NCORES)],
                       [t1r[c * pershard:(c + 1) * pershard] for c in range(NCORES)])
    DT = dec["T"]
    DBLK = DT // P

    idx_arr = {}
    for key, pr in (("l1", l1), ("l2", l2)):
        for r in range(4):
            idx_arr[(key, r)] = _wrap_idx(pr["idx16"][r])
    idx_arr[("u", 0)] = _wrap_idx(dec["idx_u"])
    idx_arr[("v", 0)] = _wrap_idx(dec["idx_v"])

    # ---------- device program ----------
    nc = bacc.Bacc("TRN2", target_bir_lowering=False, debug=False,
                   num_devices=NCORES, num_swdge_queues=4)

    def din(name, shape, dt=f16):
        return nc.dram_tensor(name, list(shape), dt, kind="ExternalInput").ap()

    xt = din("x_tab", (XTAB, P))
    w1t = din("W1r", (P, P))
    w2tt = din("W2T", (P, P))
    wcat = din("Wcat", (P, 4))
    iota_in = din("iota", (P, P))
    idx_in = {k: din(f"idx_{k[0]}_{k[1]}", v.shape[1:], i16)
              for k, v in idx_arr.items()}
    relw_in = {key: (din(f"rel_{key}", (P, pr["n_sel"]), f32),
                     din(f"w_{key}", (P, pr["n_sel"]), f32))
               for key, pr in (("l1", l1), ("l2", l2))}

    out_d = nc.dram_tensor("out_dec", [P, 2 * DBLK], f32,
                           kind="ExternalOutput").ap()
    DEBUG = bool(globals().get("DEBUG_DUMPS", False))
    if DEBUG:
        out_h = nc.dram_tensor("out_h", [SLOTS, P], f16,
                               kind="ExternalOutput").ap()
        out_uv = nc.dram_tensor("out_uv", [SLOTS, 4], f16,
                                kind="ExternalOutput").ap()
    h_slice = [nc.dram_tensor(f"h_slice{p}", [HSLOT[p], P], f16)
               for p in range(4)]
    h_tab = [nc.dram_tensor(f"h_tab{p}", [NCORES * HSLOT[p], P], f16,
                            addr_space="Shared") for p in range(4)]
    uv_slice = [nc.dram_tensor(f"uv_slice{p}", [HSLOT[p], P], f16)
                for p in range(4)]
    uv_tab = [nc.dram_tensor(f"uv_tab{p}", [NCORES * HSLOT[p], P], f16,
                             addr_space="Shared") for p in range(4)]

    qn = [0]

    def next_q():
        qn[0] = (qn[0] + 1) % 4
        return qn[0]

    with tile.TileContext(nc) as tc:
        with (
            tc.tile_pool(name="meta", bufs=1) as mp,
            tc.tile_pool(name="stageA", bufs=6) as sgpA,
            tc.tile_pool(name="stageB", bufs=6) as sgpB,
            tc.tile_pool(name="idxp", bufs=1) as ixp,
            tc.tile_pool(name="selp", bufs=12) as selp,
            tc.tile_pool(name="work", bufs=2) as wp,
            tc.tile_pool(name="dstage", bufs=1) as dsp,
            tc.tile_pool(name="psA", bufs=4, space="PSUM") as ppA,
            tc.tile_pool(name="psB", bufs=2, space="PSUM") as ppB,
            tc.tile_pool(name="psT", bufs=2, space="PSUM") as ppT,
        ):
            nc.gpsimd.load_library(mlp)
            iota_t = mp.tile([P, P], f16, name="iota_t")
            nc.sync.dma_start(iota_t[:], iota_in[:])
            ident = mp.tile([P, P], f16, name="ident")
            make_identity(nc, ident[:])
            w1_sb = mp.tile([P, P], f16, name="w1_sb")
            nc.sync.dma_start(w1_sb[:], w1t[:])
            w2t_sb = mp.tile([P, P], f16, name="w2t_sb")
            nc.sync.dma_start(w2t_sb[:], w2tt[:])
            wcat_sb = mp.tile([P, 4], f16, name="wcat_sb")
            nc.sync.dma_start(wcat_sb[:], wcat[:])
            wu_ps = ppB.tile([P, 512], f32, space="PSUM", name="wu_ps",
                             tag="psB")
            nc.tensor.matmul(wu_ps[:, 0:4], lhsT=w2t_sb[:], rhs=wcat_sb[:],
                             start=True, stop=True)
            wu_sb = mp.tile([P, 4], f16, name="wu_sb")
            nc.scalar.copy(wu_sb[:], wu_ps[:, 0:4])

            def sel_build(name, rel_sb, w_sb, col):
                sel = selp.tile([P, P], f16, name=name, tag="sel")
                nc.vector.scalar_tensor_tensor(
                    out=sel[:], in0=iota_t[:],
                    scalar=rel_sb[:, col:col + 1],
                    in1=w_sb[:, col:col + 1].to_broadcast([P, P]),
                    op0=mybir.AluOpType.is_equal,
                    op1=mybir.AluOpType.mult)
                return sel

            zero_sel = mp.tile([P, P], f16, name="zero_sel")
            nc.vector.memset(zero_sel[:], 0.0)

            def run_layer(key, pr, tab_aps, consume, split):
                """Gather + AT-route windows; consume(wi, k0, k1, at list).

                split=True: ranges {0,1} and {2,3} accumulate into separate
                PSUM banks per window (consume receives both lists) so the
                early group can run before the late ranges' tables arrive.
                """
                rel_sb = ixp.tile([P, pr["n_sel"]], f32,
                                  name=f"rel_{key}_sb", tag="relt")
                w_sb = ixp.tile([P, pr["n_sel"]], f32,
                                name=f"w_{key}_sb", tag="wt")
                nc.sync.dma_start(rel_sb[:], relw_in[key][0][:])
                nc.sync.dma_start(w_sb[:], relw_in[key][1][:])
                idx_sb = []
                for r in range(4):
                    cols = pr["layout"][r]["T"] // 16
                    it = ixp.tile([P, cols], i16, name=f"ix_{key}_{r}",
                                  tag=f"ix{r}")
                    nc.sync.dma_start(it[:], idx_in[(key, r)][:])
                    idx_sb.append(it)

                groups = [(0, 1), (2, 3)] if split else [(0, 1, 2, 3)]

                for wi in range(NW):
                    k0 = wi * W
                    k1 = min(k0 + W, CHUNKS)
                    nhalf = (k1 - k0 + 3) // 4
                    at_lists = []
                    for gi, grp in enumerate(groups):
                        stg = {}
                        for r in grp:
                            call = pr["layout"][r]["calls"][wi]
                            npad = call["n_pad"]
                            c0 = call["base"] // 16
                            pool = sgpA if r < 2 else sgpB
                            st = pool.tile([P, (npad // P) * P], f16,
                                           name=f"st_{key}_{r}_{wi}",
                                           tag=f"stage{r}")
                            nc.gpsimd.dma_gather(
                                st[:].rearrange("p (c e) -> p c e", e=P),
                                tab_aps[r], idx_sb[r][:, c0:c0 + npad // 16],
                                npad, npad, P,
                                queue_num=next_q(), single_packet=False)
                            stg[r] = st
                        at_ps = [ppA.tile([P, 512], f32, space="PSUM",
                                          name=f"at_{key}_{wi}_{gi}_{h}",
                                          tag="psA")
                                 for h in range(nhalf)]
                        cnt = {}
                        tot = {}
                        for r in grp:
                            for (b, kl, sc) in pr["sched"][(r, wi)]:
                                tot[kl] = tot.get(kl, 0) + 1
                        # PSUM accumulation groups must not interleave per
                        # region: emit each chunk's matmuls consecutively.
                        order = []
                        for r in grp:
                            for (b, kl, sc) in pr["sched"][(r, wi)]:
                                order.append((kl, r, b, sc))
                        order.sort(key=lambda t: t[0])
                        for (kl, r, b, sc) in order:
                            st = stg[r]
                            sel = sel_build(f"sel_{key}_{wi}_{r}_{b}_{kl}",
                                            rel_sb, w_sb, sc)
                            c = cnt.get(kl, 0)
                            cnt[kl] = c + 1
                            ph = at_ps[kl // 4]
                            nc.tensor.matmul(
                                ph[:, (kl % 4) * P:(kl % 4) * P + P],
                                lhsT=st[:, b * P:(b + 1) * P],
                                rhs=sel[:],
                                start=(c == 0), stop=(c + 1 == tot[kl]))
                        # chunks with no matmuls in this group: zero the slice
                        for kl in range(k1 - k0):
                            if kl not in tot:
                                ph = at_ps[kl // 4]
                                nc.tensor.matmul(
                                    ph[:, (kl % 4) * P:(kl % 4) * P + P],
                                    lhsT=stg[grp[0]][:, 0:P],
                                    rhs=zero_sel[:],
                                    start=True, stop=True)
                        at_lists.append(at_ps)
                    consume(wi, k0, k1, at_lists)

            # ---------- layer 1 ----------
            def consume_l1(wi, k0, k1, at_lists):
                for h in range((k1 - k0 + 3) // 4):
                    kk0 = k0 + 4 * h
                    kk1 = min(kk0 + 4, k1)
                    ncol = (kk1 - kk0) * P
                    at_sb = wp.tile([P, 512], f16, name=f"at1_{wi}_{h}",
                                    tag="at")
                    if len(at_lists) == 1:
                        nc.scalar.copy(at_sb[:, 0:ncol],
                                       at_lists[0][h][:, 0:ncol])
                    else:
                        ae = wp.tile([P, 512], f16, name=f"ae1_{wi}_{h}",
                                     tag="ae")
                        nc.scalar.copy(ae[:, 0:ncol],
                                       at_lists[0][h][:, 0:ncol])
                        nc.vector.tensor_tensor(
                            out=at_sb[:, 0:ncol],
                            in0=ae[:, 0:ncol],
                            in1=at_lists[1][h][:, 0:ncol],
                            op=mybir.AluOpType.add)
                    ht_ps = ppB.tile([P, 512], f32, space="PSUM",
                                     name=f"ht_{wi}_{h}", tag="psB")
                    nc.tensor.matmul(ht_ps[:, 0:ncol], lhsT=w1_sb[:],
                                     rhs=at_sb[:, 0:ncol], start=True,
                                     stop=True)
                    htr = wp.tile([P, 512], f16, name=f"htr_{wi}_{h}",
                                  tag="htr")
                    nc.scalar.activation(htr[:, 0:ncol], ht_ps[:, 0:ncol],
                                         mybir.ActivationFunctionType.Relu)
                    hrow = wp.tile([P, 4 * P], f16, name=f"hrow_{wi}_{h}",
                                   tag="hrow")
                    for j in range(kk1 - kk0):
                        tp = ppT.tile([P, P], f16, space="PSUM",
                                      name=f"tp1_{wi}_{h}_{j}", tag="psT")
                        nc.tensor.transpose(tp[:], htr[:, j * P:(j + 1) * P],
                                            ident[:])
                        nc.scalar.copy(hrow[:, j * P:(j + 1) * P], tp[:])
                    # DMA rows (chunk kk0+j) to the right piece slice
                    for j in range(kk1 - kk0):
                        k = kk0 + j
                        pc = min((k * P) // 3200, 3)
                        row0 = k * P - HS0[pc]
                        nc.sync.dma_start(
                            h_slice[pc].ap()[row0:row0 + P, :],
                            hrow[:, j * P:(j + 1) * P])
                        if DEBUG:
                            nc.sync.dma_start(
                                out_h[k * P:(k + 1) * P, :],
                                hrow[:, j * P:(j + 1) * P])

            run_layer("l1", l1, [xt[XRLO[r]:] for r in range(4)], consume_l1,
                      split=False)
            for p in range(4):
                nc.gpsimd.collective_compute(
                    "AllGather", mybir.AluOpType.bypass,
                    replica_groups=[list(range(NCORES))],
                    ins=[h_slice[p].ap()[:]], outs=[h_tab[p].ap()[:]])

            # ---------- layer 2 ----------
            def consume_l2(wi, k0, k1, at_lists):
                for h in range((k1 - k0 + 3) // 4):
                    kk0 = k0 + 4 * h
                    kk1 = min(kk0 + 4, k1)
                    ncol = (kk1 - kk0) * P
                    at_sb = wp.tile([P, 512], f16, name=f"at2_{wi}_{h}",
                                    tag="at")
                    if len(at_lists) == 1:
                        nc.scalar.copy(at_sb[:, 0:ncol],
                                       at_lists[0][h][:, 0:ncol])
                    else:
                        ae = wp.tile([P, 512], f16, name=f"ae2_{wi}_{h}",
                                     tag="ae")
                        nc.scalar.copy(ae[:, 0:ncol],
                                       at_lists[0][h][:, 0:ncol])
                        nc.vector.tensor_tensor(
                            out=at_sb[:, 0:ncol],
                            in0=ae[:, 0:ncol],
                            in1=at_lists[1][h][:, 0:ncol],
                            op=mybir.AluOpType.add)
                    uv_ps = ppB.tile([P, 512], f32, space="PSUM",
                                     name=f"uvp_{wi}_{h}", tag="psB")
                    nc.tensor.matmul(uv_ps[0:4, 0:ncol], lhsT=wu_sb[:],
                                     rhs=at_sb[:, 0:ncol], start=True,
                                     stop=True)
                    uvt = wp.tile([4, 512], f16, name=f"uvt_{wi}_{h}",
                                  tag="uvt")
                    nc.scalar.copy(uvt[:, 0:ncol], uv_ps[0:4, 0:ncol])
                    uvrow = wp.tile([P, 4 * 4], f16, name=f"uvr_{wi}_{h}",
                                    tag="uvrow")
                    for j in range(kk1 - kk0):
                        tp = ppT.tile([P, P], f16, space="PSUM",
                                      name=f"tp2_{wi}_{h}_{j}", tag="psT")
                        nc.tensor.transpose(tp[0:P, 0:4],
                                            uvt[:, j * P:(j + 1) * P],
                                            ident[0:4, 0:4])
                        nc.scalar.copy(uvrow[:, j * 4:(j + 1) * 4],
                                       tp[0:P, 0:4])
                    for j in range(kk1 - kk0):
                        k = kk0 + j
                        pc = min((k * P) // 3200, 3)
                        row0 = k * P - HS0[pc]
                        nc.sync.dma_start(
                            uv_slice[pc].ap()[row0:row0 + P, 0:4],
                            uvrow[:, j * 4:(j + 1) * 4])
                        if DEBUG:
                            nc.sync.dma_start(
                                out_uv[k * P:(k + 1) * P, :],
                                uvrow[:, j * 4:(j + 1) * 4])

            run_layer("l2", l2, [h_tab[r].ap()[:] for r in range(4)],
                      consume_l2, split=True)
            for p in range(4):
                nc.gpsimd.collective_compute(
                    "AllGather", mybir.AluOpType.bypass,
                    replica_groups=[list(range(NCORES))],
                    ins=[uv_slice[p].ap()[:]], outs=[uv_tab[p].ap()[:]])

            # ---------- decode (two halves sharing one stage pair) ----------
            iu = ixp.tile([P, DT // 16], i16, name="ix_u", tag="ixu")
            nc.sync.dma_start(iu[:], idx_in[("u", 0)][:])
            iv = ixp.tile([P, DT // 16], i16, name="ix_v", tag="ixv")
            nc.sync.dma_start(iv[:], idx_in[("v", 0)][:])
            segoff, seglen = dec["segoff"], dec["seglen"]
            res = dsp.tile([P, DBLK * 2], f32, name="res", tag="res")
            DT_A = int(segoff[8])
            DM = max(DT_A, DT - DT_A)
            for half in range(2):
                base = 0 if half == 0 else DT_A
                hlen = DT_A if half == 0 else DT - DT_A
                if hlen == 0:
                    continue
                ust = dsp.tile([P, DM], f16, name=f"ust{half}", tag="ust")
                vst = dsp.tile([P, DM], f16, name=f"vst{half}", tag="vst")
                for r0 in (0, 1) if half == 0 else (2, 3):
                    s0 = int(segoff[4 * r0])
                    n = int(segoff[4 * r0 + 4] - s0)
                    if n == 0:
                        continue
                    nc.gpsimd.dma_gather(
                        ust[:, s0 - base:s0 - base + n]
                        .rearrange("p (c e) -> p c e", e=P),
                        uv_tab[r0].ap()[:], iu[:, s0 // 16:(s0 + n) // 16],
                        n, n, P, queue_num=next_q(), single_packet=False)
                    for r1 in range(4):
                        v0 = int(segoff[4 * r0 + r1])
                        vn = int(seglen[4 * r0 + r1])
                        if vn == 0:
                            continue
                        nc.gpsimd.dma_gather(
                            vst[:, v0 - base:v0 - base + vn]
                            .rearrange("p (c e) -> p c e", e=P),
                            uv_tab[r1].ap()[:], iv[:, v0 // 16:(v0 + vn) // 16],
                            vn, vn, P, queue_num=next_q(),
                            single_packet=False)
                nblk = hlen // P
                nc.vector.tensor_tensor(
                    out=res[:, 2 * (base // P):2 * (base // P) + 2 * nblk]
                    .rearrange("p (c e) -> p c e", e=2),
                    in0=ust[:, 0:hlen].rearrange("p (c e) -> p c e",
                                                 e=P)[:, :, 0:2],
                    in1=vst[:, 0:hlen].rearrange("p (c e) -> p c e",
                                                 e=P)[:, :, 2:4],
                    op=mybir.AluOpType.add)
            nc.sync.dma_start(out_d[:], res[:])

    nc.compile()

    # ---------- stage inputs & run ----------
    iota_np = np.broadcast_to(np.arange(P, dtype=np.float16)[None, :],
                              (P, P)).copy()
    wcat_np = np.ascontiguousarray(
        np.concatenate([Wlin[:, :P].T, Wlin[:, P:].T], axis=1)).astype(np.float16)
    in_maps = []
    for c in range(NCORES):
        m = {"x_tab": x_tab, "W1r": W1.astype(np.float16),
             "W2T": np.ascontiguousarray(W2.T).astype(np.float16),
             "Wcat": wcat_np, "iota": iota_np}
        for key, pr in (("l1", l1), ("l2", l2)):
            m[f"rel_{key}"] = np.ascontiguousarray(pr["rel"][c])
            m[f"w_{key}"] = np.ascontiguousarray(pr["wgt"][c])
            for r in range(4):
                m[f"idx_{key}_{r}"] = idx_arr[(key, r)][c]
        m["idx_u_0"] = idx_arr[("u", 0)][c]
        m["idx_v_0"] = idx_arr[("v", 0)][c]
        in_maps.append(m)

    res = run_bass_kernel_spmd(nc, in_maps, core_ids=list(range(NCORES)),
                               trace=globals().get("TRACE", False))
    globals()["LAST_EXEC_NS"] = res.exec_time_ns
    globals()["LAST_RESULTS"] = res.results

    out = np.zeros((npairs, 2), np.float32)
    for c in range(NCORES):
        o3 = res.results[c]["out_dec"].reshape(P, DBLK, 2)
        pos = dec["pos_of"][c]
        out[c * pershard:(c + 1) * pershard] = o3[pos % P, pos // P]
    return out
